# revision 1
# baseline (speedup 1.0000x reference)
"""Trainium2 Bass kernel for nn_MixedHOMVector (higher-order moment pooling).

Reference computation (per batch row b, channel c, pooling over T):
    grp  = mean(x**p)            (p scalar; p==1 -> grp == mean(x))
    mu   = mean(x); var = mean((x-mu)^2)
    skew = mean((x-mu)^3) / (var+EPS)^1.5
    kurt = mean((x-mu)^4) / (var+EPS)^2
    out  = concat([grp, var, skew, kurt], -1)    # [B, 4C]

Strategy (data-parallel over batch, 8 cores, B/8 = 4 rows each):
  * Layout: tiles [128 part = t-sub, free = (s, c)]; each DMA moves a
    contiguous 1 MiB block of x.
  * Shifted power sums: y = x - 0.5. Compute y, y2, y3, y4 element-wise
    (bf16 outputs) split across DVE/ACT, then sum over t with TensorE
    ones-matmuls accumulating in PSUM (bf16 rhs = 1 col/cycle).
    Four moments go to four PE column groups (tile_position col packing)
    so their matmuls can run concurrently.
  * Central moments recovered from shifted raw sums in a tiny fp32
    epilogue done in a [c-partition, moment-free] transposed layout.
"""

import sys

if "/opt/trn_rl_repo" not in sys.path:
    sys.path.insert(0, "/opt/trn_rl_repo")

import numpy as np

B, T, C = 32, 8192, 256
N_CORES = 8
B_LOC = B // N_CORES          # batch rows per core
EPS = 1e-6
SHIFT = 0.5                   # constant shift for numerical stability

P = 128                       # SBUF partitions
TT = 1024                     # t-rows per big tile (1 MiB f32 per tile)
S = TT // P                   # t-rows per partition within a tile
FREE = S * C                  # free elements per partition per tile (2048)
NTILES = T // TT              # big tiles per batch row (8)
MMN = 512                     # matmul moving free dim (max for one PSUM bank)
NSLICE = FREE // MMN          # matmul slices per tile (4)

_CACHE = {}


def _build(p_val: float, repeat: int = 1, hwloop: int = 1, mode: str = "full",
           psum_split: bool = True, free: int = FREE):
    """Build + compile the per-core SPMD bass kernel. p_val==1.0 uses the
    fast path (grp == mean); otherwise an extra x**p = exp(p*ln x) pass.

    repeat>1 python-unrolls the main reduction loop; hwloop>1 additionally
    wraps it in a hardware For_i loop (for timing-by-slope: the wall-clock
    difference between loop counts isolates device time). Every repetition
    restarts PSUM accumulation, so the result is identical."""
    import contextlib
    import concourse.bass as bass  # noqa: F401
    import concourse.tile as tile
    from concourse import bacc, mybir
    from contextlib import ExitStack

    f32 = mybir.dt.float32
    bf16 = mybir.dt.bfloat16
    A = mybir.ActivationFunctionType
    OP = mybir.AluOpType

    p_is_one = (p_val == 1.0)
    NMOM = 4 if p_is_one else 5
    # tile geometry derived from the per-partition free size
    S_ = free // C              # t-rows per partition per tile
    TT_ = P * S_                # t-rows per tile
    NTILES_ = T // TT_          # tiles per batch row
    NSLICE_ = free // MMN       # matmul slices per tile

    nc = bacc.Bacc("TRN2", target_bir_lowering=False, debug=False,
                   num_devices=N_CORES)

    x = nc.dram_tensor("x", [B_LOC, T, C], f32, kind="ExternalInput").ap()
    out = nc.dram_tensor("out", [B_LOC, 4 * C], f32, kind="ExternalOutput").ap()
    scratch = nc.dram_tensor("scratch", [B_LOC, NMOM, MMN], f32).ap()

    # [B_LOC, NTILES, P, (s c)] view of x; per (b, j) a contiguous 1MiB block
    xv = x.rearrange("b (n p s) c -> b n p (s c)", p=P, s=S_)

    with tile.TileContext(nc) as tc, ExitStack() as ctx:
        xp = ctx.enter_context(tc.tile_pool(name="xp", bufs=6))
        yp = ctx.enter_context(tc.tile_pool(name="yp", bufs=3))
        pp = ctx.enter_context(tc.tile_pool(name="pp", bufs=4, space="PSUM"))
        sp = ctx.enter_context(tc.tile_pool(name="sp", bufs=1))
        ep = ctx.enter_context(tc.tile_pool(name="ep", bufs=1))

        ones = sp.tile([P, 1], bf16)
        nc.vector.memset(ones, 1.0)
        neg_shift = sp.tile([P, 1], f32)
        nc.vector.memset(neg_shift, -SHIFT)
        zero_b = sp.tile([P, 1], f32)
        nc.vector.memset(zero_b, 0.0)
        ident = sp.tile([P, P], f32)
        from concourse.masks import make_identity
        make_identity(nc, ident[:])
        # selector: wsel[k, n] = 1 iff k == 32n  (compacts moment rows)
        wsel = sp.tile([P, 4], f32)
        nc.gpsimd.memset(wsel, 0.0)
        nc.gpsimd.affine_select(
            out=wsel[:], in_=wsel[:],
            compare_op=mybir.AluOpType.not_equal, fill=1.0, base=0,
            pattern=[[-32, 4]], channel_multiplier=1,
        )

        # stage rows live at partitions 32*m (moment m), free = (b, s2, c)
        stage = ep.tile([P, B_LOC * MMN], f32)
        nc.gpsimd.memset(stage[:], 0.0)  # unwritten rows feed selector matmuls
        stage5 = ep.tile([1, B_LOC * MMN], f32) if not p_is_one else None

        dummy = None
        if mode == "pe":
            dummy = sp.tile([P, free], bf16, name="dummy")
            nc.vector.memset(dummy, 1.0)

        def main_block(b):
            """One batch row: reduce T into PSUM moment rows, copy to stage."""
            if psum_split:
                # one PSUM tile (= one bank) per moment, so Tile's bank
                # tracking never serializes the four column groups.
                # bufs=2 x 4 tags = exactly the 8 PSUM banks (p==1 path).
                pbufs = 2 if p_is_one else 1
                prows = [pp.tile([P, MMN], f32, tag=f"psum{m}",
                                 name=f"psum{m}", bufs=pbufs)[32 * m: 32 * m + 1, :]
                         for m in range(4)]
            else:
                psum = pp.tile([P, MMN], f32, tag="psum", name="psum")
                prows = [psum[32 * m: 32 * m + 1, :] for m in range(4)]
            psum5 = (pp.tile([1, MMN], f32, tag="psum5", name="psum5")
                     if not p_is_one else None)
            for j in range(NTILES_):
                xt = xp.tile([P, free], f32, tag="xt", name="xt")
                nc.sync.dma_start(out=xt[:], in_=xv[b, j])
                if mode == "dma":
                    continue
                if mode == "pe":
                    for k in range(NSLICE_):
                        first = (j == 0 and k == 0)
                        last = (j == NTILES_ - 1 and k == NSLICE_ - 1)
                        for m in range(4):
                            nc.tensor.matmul(
                                prows[m], ones[:],
                                dummy[:, k * MMN: (k + 1) * MMN],
                                start=first, stop=last,
                                tile_position=(0, 32 * m),
                            )
                    continue

                y1 = yp.tile([P, free], bf16, tag="y1", name="y1")
                nc.vector.tensor_scalar_add(y1[:], xt[:], -SHIFT)

                y2 = yp.tile([P, free], bf16, tag="y2", name="y2")
                nc.scalar.activation(y2[:], xt[:], A.Square,
                                     bias=neg_shift[:], scale=1.0)

                y3 = yp.tile([P, free], bf16, tag="y3", name="y3")
                nc.vector.tensor_mul(y3[:], y1[:], y2[:])

                # y4 engine split tuned so ACT and DVE finish together
                # (ACT: y2 + 3/8 of y4; DVE: y1 + y3 + 5/8 of y4)
                y4 = yp.tile([P, free], bf16, tag="y4", name="y4")
                if (b + j) % 8 < 3:
                    nc.scalar.activation(y4[:], y2[:], A.Square,
                                         bias=zero_b[:], scale=1.0)
                else:
                    nc.vector.tensor_mul(y4[:], y2[:], y2[:])

                moms = [y1, y2, y3, y4]
                if not p_is_one:
                    lnx = yp.tile([P, free], f32, tag="lnx", name="lnx")
                    nc.scalar.activation(lnx[:], xt[:], A.Log,
                                         bias=zero_b[:], scale=1.0)
                    xpw = yp.tile([P, free], bf16, tag="xpw", name="xpw")
                    nc.scalar.activation(xpw[:], lnx[:], A.Exp,
                                         bias=zero_b[:], scale=p_val)

                if mode == "nope":
                    continue
                for k in range(NSLICE_):
                    first = (j == 0 and k == 0)
                    last = (j == NTILES_ - 1 and k == NSLICE_ - 1)
                    for m, ym in enumerate(moms):
                        nc.tensor.matmul(
                            prows[m],
                            ones[:],
                            ym[:, k * MMN: (k + 1) * MMN],
                            start=first, stop=last,
                            tile_position=(0, 32 * m),
                        )
                    if not p_is_one:
                        nc.tensor.matmul(
                            psum5[:], ones[:],
                            xpw[:, k * MMN: (k + 1) * MMN],
                            start=first, stop=last,
                        )

            if mode in ("dma", "nope"):
                return
            for m in range(4):
                nc.scalar.copy(stage[32 * m: 32 * m + 1, b * MMN: (b + 1) * MMN],
                               prows[m])
            if not p_is_one:
                nc.scalar.copy(stage5[:, b * MMN: (b + 1) * MMN], psum5[:])

        def epilogue_fast(parts="all"):
            """On-chip transpose epilogue (p==1): PE-transpose the stage
            rows into [c-partition, moment] PSUM layout, 22-op fp32 math,
            PE-transpose the features back, one contiguous output DMA."""
            # selector matmuls: fold[c', (b,h,m)] = sum_s2 stage[32m, ...]
            # out = stage_block^T @ wsel  (transpose+compact+s2-fold in one)
            fold = pp.tile([P, 32], f32, tag="psum0", name="fold", bufs=2)
            for b in range(B_LOC):
                for h in range(2):
                    for s2 in range(2):
                        blk = (s2 * 2 + h) * 128
                        nc.tensor.matmul(
                            fold[:, (b * 2 + h) * 4: (b * 2 + h) * 4 + 4],
                            stage[:, b * MMN + blk: b * MMN + blk + 128],
                            wsel[:],
                            start=(s2 == 0), stop=(s2 == 1),
                        )
            if parts == "dmas":
                return

            momv = fold[:].rearrange("p (b h m) -> p m b h", b=B_LOC, h=2, m=4)
            S1, S2, S3, S4 = (momv[:, m] for m in range(4))
            invT = 1.0 / T
            G = 2 * B_LOC

            def et(name):
                return ep.tile([P, G], f32, name=name)

            # feat[p, (b, f, h)]  f = (grp, var, skew, kurt)
            feat = ep.tile([P, B_LOC * 4 * 2], f32)
            featv = feat[:].rearrange("p (b f h) -> p f b h", b=B_LOC, f=4)

            d = et("d")
            nc.vector.tensor_scalar_mul(d[:], S1, invT)           # mu - 0.5
            nc.vector.tensor_scalar(featv[:, 0], S1, invT, SHIFT,
                                    OP.mult, OP.add)              # grp = mu
            d2 = et("d2")
            nc.vector.tensor_mul(d2[:], d[:], d[:])
            nc.vector.scalar_tensor_tensor(featv[:, 1], S2, invT, d2[:],
                                           OP.mult, OP.subtract)  # var
            v, r, s0, rs0, inv3 = et("v"), et("r"), et("s0"), et("rs0"), et("inv3")
            nc.vector.tensor_scalar_add(v[:], featv[:, 1], EPS)
            nc.vector.reciprocal(r[:], v[:])                      # 1/v
            nc.scalar.activation(s0[:], v[:], A.Sqrt, bias=zero_b[:], scale=1.0)
            nc.vector.tensor_mul(rs0[:], r[:], s0[:])             # v^-1/2
            nc.vector.tensor_mul(inv3[:], rs0[:], r[:])           # v^-3/2
            # m3 = invT*(S3 - 3 d S2) + 2 d^3
            t1, u3, d32, m3 = et("t1"), et("u3"), et("d32"), et("m3")
            nc.vector.tensor_mul(t1[:], d[:], S2)
            nc.vector.scalar_tensor_tensor(u3[:], t1[:], -3.0, S3,
                                           OP.mult, OP.add)
            nc.vector.scalar_tensor_tensor(d32[:], d2[:], 2.0, d[:],
                                           OP.mult, OP.mult)      # 2 d^3
            nc.vector.scalar_tensor_tensor(m3[:], u3[:], invT, d32[:],
                                           OP.mult, OP.add)
            nc.vector.tensor_mul(featv[:, 2], m3[:], inv3[:])     # skew
            # m4 = invT*(S4 - 4 d S3 + 6 d^2 S2) - 3 d^4
            t2, u4, t3, d4m, m4, r2 = (et("t2"), et("u4"), et("t3"),
                                       et("d4m"), et("m4"), et("r2"))
            nc.vector.tensor_mul(t2[:], d[:], S3)
            nc.vector.scalar_tensor_tensor(u4[:], t2[:], -4.0, S4,
                                           OP.mult, OP.add)
            nc.vector.tensor_mul(t3[:], d2[:], S2)
            nc.vector.scalar_tensor_tensor(u4[:], t3[:], 6.0, u4[:],
                                           OP.mult, OP.add)
            nc.vector.scalar_tensor_tensor(d4m[:], d2[:], -3.0, d2[:],
                                           OP.mult, OP.mult)      # -3 d^4
            nc.vector.scalar_tensor_tensor(m4[:], u4[:], invT, d4m[:],
                                           OP.mult, OP.add)
            nc.vector.tensor_mul(r2[:], r[:], r[:])               # v^-2
            nc.vector.tensor_mul(featv[:, 3], m4[:], r2[:])       # kurt

            # transpose features -> [32 rows (b,f,h), 128 c] -> one flat DMA
            ftp = pp.tile([32, P], f32, tag="psum0", name="ftp", bufs=2)
            nc.tensor.transpose(ftp[:], feat[:], ident[:])
            fts = ep.tile([32, P], f32)
            nc.scalar.copy(fts[:], ftp[:])
            nc.sync.dma_start(out=out.rearrange("b n -> (b n)"), in_=fts[:])

        def epilogue(parts="all"):
            # stage -> scratch[b, m, (s2 c)]
            for m in range(4):
                nc.sync.dma_start(
                    out=scratch[:, m, :],
                    in_=stage[32 * m: 32 * m + 1, :].rearrange(
                        "p (b f) -> p b f", b=B_LOC),
                )
            if not p_is_one:
                nc.sync.dma_start(
                    out=scratch[:, 4, :],
                    in_=stage5[:].rearrange("p (b f) -> p b f", b=B_LOC))

            # transposed load-back: [c%128, (s2, h, b, m)]
            mom2 = ep.tile([P, 2 * 2 * B_LOC * NMOM], f32)
            scr_v = scratch.rearrange("b m (s2 h p) -> s2 h p b m", s2=2, h=2, p=P)
            mom2_v = mom2[:].rearrange("p (s2 h b m) -> p s2 h b m",
                                       s2=2, h=2, b=B_LOC, m=NMOM)
            for s2 in range(2):
                for h in range(2):
                    nc.sync.dma_start(out=mom2_v[:, s2, h], in_=scr_v[s2, h])
            # fold s-parity: mom[p, (h, b, m)]
            NF = 2 * B_LOC * NMOM
            mom = ep.tile([P, NF], f32)
            nc.vector.tensor_add(mom[:], mom2[:, 0:NF], mom2[:, NF:2 * NF])
            if parts == "dmas":
                return

            # per-moment [P, (h, b)] views
            momv = mom[:].rearrange("p (h b m) -> p m (h b)", h=2, b=B_LOC, m=NMOM)
            invT = 1.0 / T
            G = 2 * B_LOC  # free size of one moment slice

            def et(name):
                return ep.tile([P, G], f32, name=name)

            d, e2, e3, e4 = et("d"), et("e2"), et("e3"), et("e4")
            nc.vector.tensor_scalar_mul(d[:], momv[:, 0], invT)   # mean(y) = mu-0.5
            nc.vector.tensor_scalar_mul(e2[:], momv[:, 1], invT)
            nc.vector.tensor_scalar_mul(e3[:], momv[:, 2], invT)
            nc.vector.tensor_scalar_mul(e4[:], momv[:, 3], invT)

            # feat[p, (h, b, f)]  f = (grp, var, skew, kurt)
            feat = ep.tile([P, 2 * B_LOC * 4], f32)
            featv = feat[:].rearrange("p (h b f) -> p f (h b)", h=2, b=B_LOC, f=4)

            if p_is_one:
                nc.vector.tensor_scalar_add(featv[:, 0], d[:], SHIFT)   # grp = mu
            else:
                nc.vector.tensor_scalar_mul(featv[:, 0], momv[:, 4], invT)

            d2 = et("d2")
            nc.vector.tensor_mul(d2[:], d[:], d[:])
            nc.vector.tensor_sub(featv[:, 1], e2[:], d2[:])             # var
            d3, t1, m3 = et("d3"), et("t1"), et("m3")
            nc.vector.tensor_mul(d3[:], d2[:], d[:])
            nc.vector.tensor_mul(t1[:], d[:], e2[:])
            nc.vector.scalar_tensor_tensor(m3[:], t1[:], -3.0, e3[:], OP.mult, OP.add)
            nc.vector.scalar_tensor_tensor(m3[:], d3[:], 2.0, m3[:], OP.mult, OP.add)
            t2, t3, d4, m4 = et("t2"), et("t3"), et("d4"), et("m4")
            nc.vector.tensor_mul(t2[:], d[:], e3[:])
            nc.vector.scalar_tensor_tensor(m4[:], t2[:], -4.0, e4[:], OP.mult, OP.add)
            nc.vector.tensor_mul(t3[:], d2[:], e2[:])
            nc.vector.scalar_tensor_tensor(m4[:], t3[:], 6.0, m4[:], OP.mult, OP.add)
            nc.vector.tensor_mul(d4[:], d2[:], d2[:])
            nc.vector.scalar_tensor_tensor(m4[:], d4[:], -3.0, m4[:], OP.mult, OP.add)

            # v = var + EPS; rstd via ACT sqrt + accurate reciprocal + 2 Newton
            v, s0, r, tn = et("v"), et("s0"), et("r"), et("tn")
            nc.vector.tensor_scalar_add(v[:], featv[:, 1], EPS)
            nc.scalar.activation(s0[:], v[:], A.Sqrt, bias=zero_b[:], scale=1.0)
            nc.vector.reciprocal(r[:], v[:])
            nc.vector.tensor_mul(tn[:], s0[:], r[:])                    # ~ v^-1/2
            tsq, w, u = et("tsq"), et("w"), et("u")
            for _ in range(1):  # Newton: t = t*(1.5 - 0.5*v*t^2)
                nc.vector.tensor_mul(tsq[:], tn[:], tn[:])
                nc.vector.tensor_mul(w[:], v[:], tsq[:])
                nc.vector.scalar_tensor_tensor(u[:], w[:], -0.5, tn[:], OP.mult, OP.mult)
                nc.vector.scalar_tensor_tensor(tn[:], tn[:], 1.5, u[:], OP.mult, OP.add)
            inv3, r2 = et("inv3"), et("r2")
            nc.vector.tensor_mul(inv3[:], tn[:], tn[:])
            nc.vector.tensor_mul(inv3[:], inv3[:], tn[:])               # v^-1.5
            nc.vector.tensor_mul(featv[:, 2], m3[:], inv3[:])           # skew
            nc.vector.tensor_mul(r2[:], r[:], r[:])                     # v^-2
            nc.vector.tensor_mul(featv[:, 3], m4[:], r2[:])             # kurt

            out_v = out.rearrange("b (f h p) -> h b p f", f=4, h=2, p=P)
            feat_v = feat[:].rearrange("p (h b f) -> p h b f", h=2, b=B_LOC, f=4)
            for h in range(2):
                for b in range(B_LOC):
                    nc.sync.dma_start(out=out_v[h, b], in_=feat_v[:, h, b])


        if mode.startswith("fullep"):
            epi = epilogue_fast if p_is_one else epilogue
            with tc.For_i(0, hwloop, 1):
                for _ in range(repeat):
                    for b in range(B_LOC):
                        main_block(b)
                epi("dmas" if mode == "fullepB" else "all")
            if mode == "fullepB":
                epi()   # real epilogue once outside so out is written
        else:
            loop_cm = (tc.For_i(0, hwloop, 1) if hwloop > 1
                       else contextlib.nullcontext())
            with loop_cm:
                for _ in range(repeat):
                    for b in range(B_LOC):
                        main_block(b)
            if mode not in ("dma", "nope"):
                (epilogue_fast if p_is_one else epilogue)()

    nc.compile()
    return nc


def _get(p_val: float, repeat: int = 1, hwloop: int = 1, mode: str = "full",
         psum_split: bool = True, free: int = FREE):
    key = (p_val, repeat, hwloop, mode, psum_split, free)
    if key not in _CACHE:
        _CACHE[key] = _build(p_val, repeat, hwloop, mode, psum_split, free)
    return _CACHE[key]


def run_sharded(x, p, trace=False, repeat=1, hwloop=1, **kw):
    """Run the SPMD kernel on 8 cores. Returns (out [B,4C], BassKernelResults)."""
    from concourse.bass_utils import run_bass_kernel_spmd

    x = np.ascontiguousarray(np.asarray(x, dtype=np.float32))
    assert x.shape == (B, T, C), x.shape
    p_val = float(np.asarray(p).reshape(-1)[0])
    nc = _get(p_val, repeat, hwloop)
    in_maps = [{"x": x[i * B_LOC:(i + 1) * B_LOC]} for i in range(N_CORES)]
    res = run_bass_kernel_spmd(nc, in_maps, core_ids=list(range(N_CORES)),
                               trace=trace, **kw)
    outp = np.concatenate([r["out"] for r in res.results], axis=0)
    return outp, res


def kernel(x, p):
    return run_sharded(x, p)[0]

